# revision 1
# baseline (speedup 1.0000x reference)
"""Trainium2 Bass kernel: CrossframeLocalInterpolationModule (gnn message passing).

Computation per vertex n (N=500000, C=32, K=9):
  neigh  = hidden_state[safe_idx]                (masked gather)
  dist_k = ||neigh_k - lv_n||_2 * valid_k
  dist_n = dist / sum_k dist
  w_k    = relu(alpha - dist_n) * beta * valid_k
  aflow  = sum_k w_k * neigh_k + b_aflow
  out    = relu([aflow, lv] @ W + b_lin)

Sharding: vertices split evenly over 8 cores (data parallel); hidden_state
table + the tiny linear weights replicated per core. Neighbor gather is done
with one indirect DMA per tile of 1024 vertices.

Identity used on-device: sum_k w_k*neigh_k = sum_k w_k*(neigh_k - lv) + (sum_k w_k)*lv
so the gathered tile can be destroyed in-place by the diff computation.
"""

import math
import numpy as np

try:
    import concourse.bass as bass
except ImportError:  # pragma: no cover - fallback path
    import sys

    sys.path.insert(0, "/opt/trn_rl_repo")
    import concourse.bass as bass

import concourse.bacc as bacc

from contextlib import ExitStack

import concourse.tile as tile_mod
from concourse import mybir
from concourse.bass_utils import run_bass_kernel_spmd
from concourse.masks import make_identity

F32 = mybir.dt.float32
I32 = mybir.dt.int32
ALU = mybir.AluOpType
ACTF = mybir.ActivationFunctionType
AX = mybir.AxisListType

N_FULL = 500000
C = 32
K = 9
NCORES = 8
P = 128
T_MAIN = 8  # 128-vertex sub-tiles per big tile

# pad so every core gets an equal whole number of 128-vertex sub-tiles
PER_CORE = 62592  # = 489 * 128 ;  8 * 62592 = 500736 >= 500000
PAD_N = PER_CORE * NCORES


def _subtile_plan(per_core, t_main):
    s = per_core // P
    tiles = [t_main] * (s // t_main)
    if s % t_main:
        tiles.append(s % t_main)
    return tiles


def _ap(base, dims):
    """Build an AP on the same tensor as `base` ([P, free...] tile view) with
    custom free dims [[step, count], ...] (element units)."""
    return bass.AP(
        tensor=base.tensor,
        offset=base.offset,
        ap=[list(base.ap[0])] + [list(d) for d in dims],
    )


def build_program(per_core, table_rows, alpha, beta, t_main=T_MAIN):
    nc = bacc.Bacc()

    lv_d = nc.dram_tensor("lv", [per_core, C], F32, kind="ExternalInput")
    hs_d = nc.dram_tensor("hs", [table_rows, C], F32, kind="ExternalInput")
    idx_d = nc.dram_tensor("nidx", [per_core, K], I32, kind="ExternalInput")
    # rows 0:64 = W, row 64 = b_lin  (bias via ones-column trick)
    wb_d = nc.dram_tensor("wb", [2 * C + 1, C], F32, kind="ExternalInput")
    out_d = nc.dram_tensor("out", [per_core, C], F32, kind="ExternalOutput")

    tiles = _subtile_plan(per_core, t_main)

    with ExitStack() as ctx:
        tc = ctx.enter_context(tile_mod.TileContext(nc))
        singles = ctx.enter_context(tc.tile_pool(name="singles", bufs=1))
        ident = singles.tile([P, P], F32)
        make_identity(nc, ident[:])
        wb_sb = singles.tile([2 * C + 1, C], F32)
        nc.sync.dma_start(out=wb_sb[:], in_=wb_d[:, :])
        alpha_t = singles.tile([P, 1], F32)
        nc.vector.memset(alpha_t[:], float(alpha))

        gpool = ctx.enter_context(tc.tile_pool(name="gpool", bufs=2))
        sqpool = ctx.enter_context(tc.tile_pool(name="sqpool", bufs=2))
        catpool = ctx.enter_context(tc.tile_pool(name="catpool", bufs=2))
        idxpool = ctx.enter_context(tc.tile_pool(name="idxpool", bufs=3))
        statpool = ctx.enter_context(tc.tile_pool(name="statpool", bufs=2))
        outpool = ctx.enter_context(tc.tile_pool(name="outpool", bufs=2))
        ctpool = ctx.enter_context(tc.tile_pool(name="ctpool", bufs=3))
        tps = ctx.enter_context(tc.tile_pool(name="tps", bufs=2, space="PSUM"))
        mps = ctx.enter_context(tc.tile_pool(name="mps", bufs=2, space="PSUM"))

        base = 0
        for tile_i, T in enumerate(tiles):
            _emit_tile(
                nc,
                pools=dict(
                    gpool=gpool,
                    sqpool=sqpool,
                    catpool=catpool,
                    idxpool=idxpool,
                    statpool=statpool,
                    outpool=outpool,
                    ctpool=ctpool,
                    tps=tps,
                    mps=mps,
                ),
                ident=ident,
                wb_sb=wb_sb,
                alpha_t=alpha_t,
                lv_d=lv_d,
                hs_d=hs_d,
                idx_d=idx_d,
                out_d=out_d,
                base=base,
                T=T,
                beta=beta,
                t_main=t_main,
                tile_i=tile_i,
            )
            base += T * P

    nc.compile()
    return nc


def _strip_redundant_dma_waits(nc):
    """Walrus allows at most 2 sync waits on a DMA instruction. For the
    idx_raw loads, the DVE slot-release wait transitively dominates any
    DMAHW lane waits (every idx DMA is RAW-waited by DVE before the release
    tick), so lane waits beyond the limit can be dropped safely."""
    for f in nc.m.functions:
        for blk in f.blocks:
            for inst in blk.instructions:
                if type(inst).__name__ != "InstDMACopy":
                    continue
                si = inst.sync_info
                if si is None or len(si.on_wait or []) <= 2:
                    continue
                waits = list(si.on_wait)
                keep = [w for w in waits if not w.ant_name.startswith("DMAHW")]
                has_compute = any(
                    not w.ant_name.startswith(("DMAHW", "DMASW")) for w in keep
                )
                if has_compute and len(keep) <= 2:
                    si.on_wait = keep


def _emit_tile(nc, pools, ident, wb_sb, alpha_t, lv_d, hs_d, idx_d, out_d, base, T, beta, t_main, tile_i=0):
    KT = T * K
    F = T * K * C
    rows = T * P
    CAT = 2 * C + 1  # 65

    gpool = pools["gpool"]
    sqpool = pools["sqpool"]
    catpool = pools["catpool"]
    idxpool = pools["idxpool"]
    statpool = pools["statpool"]
    outpool = pools["outpool"]
    ctpool = pools["ctpool"]
    tps = pools["tps"]
    mps = pools["mps"]

    # vertex mapping within the tile: v = base + p * T + t
    # ---- load neighbor indices ----
    idx_raw = idxpool.tile([P, t_main * K], I32, tag="idx_raw")
    nc.scalar.dma_start(
        out=idx_raw[:, :KT],
        in_=idx_d[base : base + rows, :].rearrange("(p t) k -> p (t k)", t=T),
    )
    idx_safe = idxpool.tile([P, t_main * K], I32, tag="idx_safe")
    nc.vector.tensor_scalar_max(idx_safe[:, :KT], idx_raw[:, :KT], 0)
    # valid-mask * beta  (f32)
    vmb = idxpool.tile([P, t_main * K], F32, tag="vmb")
    nc.vector.tensor_scalar(
        out=vmb[:, :KT],
        in0=idx_raw[:, :KT],
        scalar1=0,
        scalar2=float(beta),
        op0=ALU.is_ge,
        op1=ALU.mult,
    )

    # ---- gather neighbors ----
    # HW indirect DMA semantics: one descriptor per partition, one index per
    # partition (idx[p, 0]) transferring the whole per-partition out row
    # contiguously. So gather one (t, k) slot per instruction: idx [P, 1],
    # out [P, C].
    gbuf = gpool.tile([P, t_main * K * C], F32, tag="gbuf")
    for j in range(KT):
        nc.gpsimd.indirect_dma_start(
            out=gbuf[:, j * C : (j + 1) * C],
            out_offset=None,
            in_=hs_d[:, :],
            in_offset=bass.IndirectOffsetOnAxis(ap=idx_safe[:, j : j + 1], axis=0),
        )

    # ---- lv load (contiguous per partition) + cat tile ----
    lvb = catpool.tile([P, t_main * C], F32, tag="lvb")
    nc.scalar.dma_start(
        out=lvb[:, : T * C],
        in_=lv_d[base : base + rows, :].rearrange("(p t) c -> p (t c)", t=T),
    )
    cat = catpool.tile([P, t_main, 2 * C + 3], F32, tag="cat")
    catw = 2 * C + 3
    nc.scalar.copy(
        out=cat[:, :T, C : 2 * C],
        in_=lvb[:, : T * C].rearrange("p (t c) -> p t c", t=T),
    )
    nc.vector.memset(cat[:, :T, 2 * C : 2 * C + 1], 1.0)

    # ---- diff = neigh - lv (lv broadcast read from cat so lvb stays ACT-only) ----
    g4 = gbuf[:, :F].rearrange("p (t k c) -> p t k c", t=T, k=K)
    dbuf = gpool.tile([P, t_main * K * C], F32, tag="dbuf")
    d4 = dbuf[:, :F].rearrange("p (t k c) -> p t k c", t=T, k=K)
    cat_base = cat[:, :, :]
    lv_bc = bass.AP(
        tensor=cat_base.tensor,
        offset=cat_base.offset + C,
        ap=[list(cat_base.ap[0]), [catw, T], [0, K], [1, C]],
    )
    nc.vector.tensor_tensor(out=d4, in0=g4, in1=lv_bc, op=ALU.subtract)

    # ---- squared distance ----
    sq = sqpool.tile([P, t_main * K * C], F32, tag="sq")
    nc.scalar.square(sq[:, :F], dbuf[:, :F])
    dsq = statpool.tile([P, t_main * K], F32, tag="dsq")
    nc.vector.tensor_reduce(
        out=dsq[:, :KT],
        in_=sq[:, :F].rearrange("p (tk c) -> p tk c", c=C),
        axis=AX.X,
        op=ALU.add,
    )
    dist = statpool.tile([P, t_main * K], F32, tag="dist")
    nc.scalar.sqrt(dist[:, :KT], dsq[:, :KT])

    # ---- masked dist, -sum, -1/sum ----
    mdist = statpool.tile([P, t_main * K], F32, tag="mdist")
    nc.vector.tensor_mul(mdist[:, :KT], dist[:, :KT], vmb[:, :KT])
    nssum = statpool.tile([P, t_main], F32, tag="nssum")
    nc.vector.tensor_reduce(
        out=nssum[:, :T],
        in_=mdist[:, :KT].rearrange("p (t k) -> p t k", k=K),
        axis=AX.X,
        op=ALU.add,
        negate=True,
    )
    nrecip = statpool.tile([P, t_main], F32, tag="nrecip")
    nc.vector.reciprocal(nrecip[:, :T], nssum[:, :T])

    # ---- w = relu(alpha - mdist/S) * vmb ----
    w = statpool.tile([P, t_main * K], F32, tag="w")
    for t in range(T):
        nc.scalar.activation(
            out=w[:, t * K : (t + 1) * K],
            in_=mdist[:, t * K : (t + 1) * K],
            func=ACTF.Relu,
            bias=alpha_t[:, :],
            scale=nrecip[:, t : t + 1],
        )
    nc.vector.tensor_mul(w[:, :KT], w[:, :KT], vmb[:, :KT])

    wsum = statpool.tile([P, t_main], F32, tag="wsum")
    nc.vector.tensor_reduce(
        out=wsum[:, :T],
        in_=w[:, :KT].rearrange("p (t k) -> p t k", k=K),
        axis=AX.X,
        op=ALU.add,
    )

    # ---- wdiff = diff * w (in place), reduce over k ----
    w_bc = w[:, :KT].rearrange("p (t k) -> p t k", k=K).to_broadcast((P, T, K, C))
    nc.vector.tensor_tensor(out=d4, in0=d4, in1=w_bc, op=ALU.mult)
    wdsum = statpool.tile([P, t_main * C], F32, tag="wdsum")
    g_kred = _ap(dbuf[:], [[K * C, T], [1, C], [C, K]])
    nc.vector.tensor_reduce(out=wdsum[:, : T * C], in_=g_kred, axis=AX.X, op=ALU.add)

    # ---- aflow = wdsum + wsum * lv  -> cat[:, t, 0:C] ----
    for t in range(T):
        nc.vector.scalar_tensor_tensor(
            out=cat[:, t, 0:C],
            in0=cat[:, t, C : 2 * C],
            scalar=wsum[:, t : t + 1],
            in1=wdsum[:, t * C : (t + 1) * C],
            op0=ALU.mult,
            op1=ALU.add,
        )

    # ---- linear layer + relu per sub-tile ----
    outsb = outpool.tile([P, t_main * C], F32, tag="outsb")
    for t in range(T):
        ctps = tps.tile([CAT, P], F32, tag="ctps")
        nc.tensor.transpose(out=ctps[:], in_=cat[:, t, 0:CAT], identity=ident[:])
        ctsb = ctpool.tile([CAT, P], F32, tag="ctsb")
        nc.scalar.copy(ctsb[:], ctps[:])
        ops = mps.tile([P, C], F32, tag="ops")
        nc.tensor.matmul(out=ops[:], lhsT=ctsb[:], rhs=wb_sb[:], start=True, stop=True)
        nc.scalar.activation(out=outsb[:, t * C : (t + 1) * C], in_=ops[:], func=ACTF.Relu)

    nc.scalar.dma_start(
        out=out_d[base : base + rows, :].rearrange("(p t) c -> p (t c)", t=T),
        in_=outsb[:, : T * C],
    )


_PROGRAM_CACHE = {}


def _get_program(per_core, table_rows, alpha, beta, t_main=T_MAIN):
    key = (per_core, table_rows, float(alpha), float(beta), t_main)
    if key not in _PROGRAM_CACHE:
        _PROGRAM_CACHE[key] = build_program(per_core, table_rows, alpha, beta, t_main)
    return _PROGRAM_CACHE[key]


def _shard_inputs(lv, hidden_state, W, b_lin, b_aflow, alpha, beta, neighbor_idx):
    """Pad + shard on host. Returns in_maps for the 8 cores."""
    lv = np.ascontiguousarray(np.asarray(lv, dtype=np.float32))
    hs = np.ascontiguousarray(np.asarray(hidden_state, dtype=np.float32))
    idx = np.ascontiguousarray(np.asarray(neighbor_idx, dtype=np.int32))
    W = np.asarray(W, dtype=np.float32)
    b_lin = np.asarray(b_lin, dtype=np.float32)
    b_aflow = np.asarray(b_aflow, dtype=np.float32)

    n = lv.shape[0]
    pad = PAD_N - n
    lv_p = np.concatenate([lv, np.zeros((pad, C), np.float32)], axis=0)
    idx_p = np.concatenate([idx, np.zeros((pad, K), np.int32)], axis=0)

    # fold b_aflow into the linear layer: aflow' = aflow_nobias, and
    # cat @ W + b_lin == [aflow', lv, 1] @ [[W],[b_lin + b_aflow @ W_a]]
    # where W_a is the first C rows of W (the aflow part).
    bias_row = b_lin + b_aflow @ W[:C, :]
    wb = np.concatenate([W, bias_row[None, :]], axis=0).astype(np.float32)

    in_maps = []
    for i in range(NCORES):
        s = i * PER_CORE
        e = s + PER_CORE
        in_maps.append(
            {
                "lv": lv_p[s:e],
                "hs": hs,
                "nidx": idx_p[s:e],
                "wb": wb,
            }
        )
    return in_maps


def kernel(lv, hidden_state, W, b_lin, b_aflow, alpha, beta, neighbor_idx):
    n = np.asarray(lv).shape[0]
    in_maps = _shard_inputs(lv, hidden_state, W, b_lin, b_aflow, alpha, beta, neighbor_idx)
    nc = _get_program(PER_CORE, np.asarray(hidden_state).shape[0], float(alpha), float(beta))
    res = run_bass_kernel_spmd(nc, in_maps, core_ids=list(range(NCORES)))
    out = np.concatenate([res.results[i]["out"] for i in range(NCORES)], axis=0)
    return out[:n]



# revision 7
# speedup vs baseline: 3.6009x; 3.6009x over previous
"""Trainium2 Bass kernel: CrossframeLocalInterpolationModule (gnn message passing).

Computation per vertex n (N=500000, C=32, K=9):
  neigh  = hidden_state[safe_idx]                (masked gather)
  dist_k = ||neigh_k - lv_n||_2 * valid_k
  dist_n = dist / sum_k dist
  w_k    = relu(alpha - dist_n) * beta * valid_k
  aflow  = sum_k w_k * neigh_k + b_aflow
  out    = relu([aflow, lv] @ W + b_lin)

Sharding: vertices split evenly over 8 cores (data parallel); hidden_state
table + the tiny linear weights replicated per core. Neighbor gather is done
with one indirect DMA per tile of 1024 vertices.

Identity used on-device: sum_k w_k*neigh_k = sum_k w_k*(neigh_k - lv) + (sum_k w_k)*lv
so the gathered tile can be destroyed in-place by the diff computation.
"""

import math
import numpy as np

try:
    import concourse.bass as bass
except ImportError:  # pragma: no cover - fallback path
    import sys

    sys.path.insert(0, "/opt/trn_rl_repo")
    import concourse.bass as bass

import concourse.bacc as bacc

from contextlib import ExitStack

import concourse.tile as tile_mod
from concourse import mybir
from concourse.bass_utils import run_bass_kernel_spmd
from concourse.masks import make_identity

F32 = mybir.dt.float32
I32 = mybir.dt.int32
ALU = mybir.AluOpType
ACTF = mybir.ActivationFunctionType
AX = mybir.AxisListType

N_FULL = 500000
C = 32
K = 9
NCORES = 8
P = 128
T_MAIN = 8  # 128-vertex sub-tiles per big tile

# pad so every core gets an equal whole number of 128-vertex sub-tiles
PER_CORE = 62592  # = 489 * 128 ;  8 * 62592 = 500736 >= 500000
PAD_N = PER_CORE * NCORES


def _subtile_plan(per_core, t_main):
    s = per_core // P
    tiles = [t_main] * (s // t_main)
    if s % t_main:
        tiles.append(s % t_main)
    return tiles


def _ap(base, dims):
    """Build an AP on the same tensor as `base` ([P, free...] tile view) with
    custom free dims [[step, count], ...] (element units)."""
    return bass.AP(
        tensor=base.tensor,
        offset=base.offset,
        ap=[list(base.ap[0])] + [list(d) for d in dims],
    )


def build_program(per_core, table_rows, alpha, beta, t_main=T_MAIN):
    nc = bacc.Bacc()

    lv_d = nc.dram_tensor("lv", [per_core, C], F32, kind="ExternalInput")
    hs_d = nc.dram_tensor("hs", [table_rows, C], F32, kind="ExternalInput")
    idx_d = nc.dram_tensor("nidx", [per_core, K], I32, kind="ExternalInput")
    # rows 0:64 = W, row 64 = b_lin  (bias via ones-column trick)
    wb_d = nc.dram_tensor("wb", [2 * C + 1, C], F32, kind="ExternalInput")
    out_d = nc.dram_tensor("out", [per_core, C], F32, kind="ExternalOutput")

    tiles = _subtile_plan(per_core, t_main)

    with ExitStack() as ctx:
        tc = ctx.enter_context(tile_mod.TileContext(nc))
        singles = ctx.enter_context(tc.tile_pool(name="singles", bufs=1))
        ident = singles.tile([P, P], F32)
        make_identity(nc, ident[:])
        wb_sb = singles.tile([2 * C + 1, C], F32)
        nc.sync.dma_start(out=wb_sb[:], in_=wb_d[:, :])
        alpha_t = singles.tile([P, 1], F32)
        nc.vector.memset(alpha_t[:], float(alpha))

        # SWDGE drain-buffer scratch: a junk SBUF->SBUF copy issued before each
        # gather keeps all 16 SDMA engines busy behind Q7 descriptor
        # generation, so they never race the ring writes (torn descriptors).
        dummy_src = singles.tile([P, 2048], F32)
        nc.vector.memset(dummy_src[:], 0.0)
        dpool = ctx.enter_context(tc.tile_pool(name="dpool", bufs=2))

        gpool = ctx.enter_context(tc.tile_pool(name="gpool", bufs=2))
        sqpool = ctx.enter_context(tc.tile_pool(name="sqpool", bufs=2))
        catpool = ctx.enter_context(tc.tile_pool(name="catpool", bufs=2))
        idxpool = ctx.enter_context(tc.tile_pool(name="idxpool", bufs=3))
        statpool = ctx.enter_context(tc.tile_pool(name="statpool", bufs=2))
        outpool = ctx.enter_context(tc.tile_pool(name="outpool", bufs=2))
        ctpool = ctx.enter_context(tc.tile_pool(name="ctpool", bufs=3))
        tps = ctx.enter_context(tc.tile_pool(name="tps", bufs=2, space="PSUM"))
        mps = ctx.enter_context(tc.tile_pool(name="mps", bufs=2, space="PSUM"))

        base = 0
        for tile_i, T in enumerate(tiles):
            _emit_tile(
                nc,
                pools=dict(
                    dpool=dpool,
                    dummy_src=dummy_src,
                    gpool=gpool,
                    sqpool=sqpool,
                    catpool=catpool,
                    idxpool=idxpool,
                    statpool=statpool,
                    outpool=outpool,
                    ctpool=ctpool,
                    tps=tps,
                    mps=mps,
                ),
                ident=ident,
                wb_sb=wb_sb,
                alpha_t=alpha_t,
                lv_d=lv_d,
                hs_d=hs_d,
                idx_d=idx_d,
                out_d=out_d,
                base=base,
                T=T,
                beta=beta,
                t_main=t_main,
                tile_i=tile_i,
            )
            base += T * P

    nc.compile()
    return nc


def _strip_redundant_dma_waits(nc):
    """Walrus allows at most 2 sync waits on a DMA instruction. For the
    idx_raw loads, the DVE slot-release wait transitively dominates any
    DMAHW lane waits (every idx DMA is RAW-waited by DVE before the release
    tick), so lane waits beyond the limit can be dropped safely."""
    for f in nc.m.functions:
        for blk in f.blocks:
            for inst in blk.instructions:
                if type(inst).__name__ != "InstDMACopy":
                    continue
                si = inst.sync_info
                if si is None or len(si.on_wait or []) <= 2:
                    continue
                waits = list(si.on_wait)
                keep = [w for w in waits if not w.ant_name.startswith("DMAHW")]
                has_compute = any(
                    not w.ant_name.startswith(("DMAHW", "DMASW")) for w in keep
                )
                if has_compute and len(keep) <= 2:
                    si.on_wait = keep


def _emit_tile(nc, pools, ident, wb_sb, alpha_t, lv_d, hs_d, idx_d, out_d, base, T, beta, t_main, tile_i=0):
    KT = T * K
    F = T * K * C
    rows = T * P
    CAT = 2 * C + 1  # 65

    gpool = pools["gpool"]
    sqpool = pools["sqpool"]
    catpool = pools["catpool"]
    idxpool = pools["idxpool"]
    statpool = pools["statpool"]
    outpool = pools["outpool"]
    ctpool = pools["ctpool"]
    tps = pools["tps"]
    mps = pools["mps"]

    # vertex mapping within the tile: v = base + p * T + t
    # ---- load neighbor indices ----
    idx_raw = idxpool.tile([P, t_main * K], I32, tag="idx_raw")
    nc.scalar.dma_start(
        out=idx_raw[:, :KT],
        in_=idx_d[base : base + rows, :].rearrange("(p t) k -> p (t k)", t=T),
    )
    idx_safe = idxpool.tile([P, t_main * K], I32, tag="idx_safe")
    nc.vector.tensor_scalar_max(idx_safe[:, :KT], idx_raw[:, :KT], 0)
    # valid-mask * beta  (f32)
    vmb = idxpool.tile([P, t_main * K], F32, tag="vmb")
    nc.vector.tensor_scalar(
        out=vmb[:, :KT],
        in0=idx_raw[:, :KT],
        scalar1=0,
        scalar2=float(beta),
        op0=ALU.is_ge,
        op1=ALU.mult,
    )

    # ---- gather neighbors ----
    # One batched indirect DMA per tile: offset AP [P, KT] -> P*KT descriptors
    # of C*4 bytes each. SWDGE cost is 994ns fixed + 0.34ns/descriptor, so
    # batching all KT slots into one instruction amortizes the fixed cost.
    gbuf = gpool.tile([P, t_main * K * C], F32, tag="gbuf")
    # Sacrificial SWDGE copy: its 128 descriptors keep the 16 SDMA engines
    # chewing junk while Q7 writes the gather descriptors below, so the ring
    # reads can't race the writes.
    dummy_dst = pools["dpool"].tile([P, 2048], F32, tag="dummy_dst")
    nc.gpsimd.dma_start(out=dummy_dst[:], in_=pools["dummy_src"][:])
    # 24 slots -> 3072 descriptor pairs per instruction bounds ring occupancy.
    max_slots = 24
    for s0 in range(0, KT, max_slots):
        s1 = min(s0 + max_slots, KT)
        nc.gpsimd.indirect_dma_start(
            out=gbuf[:, s0 * C : s1 * C].rearrange("p (tk c) -> p tk c", c=C),
            out_offset=None,
            in_=hs_d[:, :],
            in_offset=bass.IndirectOffsetOnAxis(ap=idx_safe[:, s0:s1], axis=0),
        )

    # ---- lv load (contiguous per partition) + cat tile ----
    lvb = catpool.tile([P, t_main * C], F32, tag="lvb")
    nc.scalar.dma_start(
        out=lvb[:, : T * C],
        in_=lv_d[base : base + rows, :].rearrange("(p t) c -> p (t c)", t=T),
    )
    cat = catpool.tile([P, t_main, 2 * C + 3], F32, tag="cat")
    catw = 2 * C + 3
    nc.scalar.copy(
        out=cat[:, :T, C : 2 * C],
        in_=lvb[:, : T * C].rearrange("p (t c) -> p t c", t=T),
    )
    nc.vector.memset(cat[:, :T, 2 * C : 2 * C + 1], 1.0)

    # ---- diff = neigh - lv (lv broadcast read from cat so lvb stays ACT-only) ----
    g4 = gbuf[:, :F].rearrange("p (t k c) -> p t k c", t=T, k=K)
    dbuf = gpool.tile([P, t_main * K * C], F32, tag="dbuf")
    d4 = dbuf[:, :F].rearrange("p (t k c) -> p t k c", t=T, k=K)
    cat_base = cat[:, :, :]
    lv_bc = bass.AP(
        tensor=cat_base.tensor,
        offset=cat_base.offset + C,
        ap=[list(cat_base.ap[0]), [catw, T], [0, K], [1, C]],
    )
    nc.vector.tensor_tensor(out=d4, in0=g4, in1=lv_bc, op=ALU.subtract)

    # ---- squared distance ----
    sq = sqpool.tile([P, t_main * K * C], F32, tag="sq")
    nc.scalar.square(sq[:, :F], dbuf[:, :F])
    dsq = statpool.tile([P, t_main * K], F32, tag="dsq")
    nc.vector.tensor_reduce(
        out=dsq[:, :KT],
        in_=sq[:, :F].rearrange("p (tk c) -> p tk c", c=C),
        axis=AX.X,
        op=ALU.add,
    )
    dist = statpool.tile([P, t_main * K], F32, tag="dist")
    nc.scalar.sqrt(dist[:, :KT], dsq[:, :KT])

    # ---- masked dist, -sum, -1/sum ----
    mdist = statpool.tile([P, t_main * K], F32, tag="mdist")
    nc.vector.tensor_mul(mdist[:, :KT], dist[:, :KT], vmb[:, :KT])
    nssum = statpool.tile([P, t_main], F32, tag="nssum")
    nc.vector.tensor_reduce(
        out=nssum[:, :T],
        in_=mdist[:, :KT].rearrange("p (t k) -> p t k", k=K),
        axis=AX.X,
        op=ALU.add,
        negate=True,
    )
    nrecip = statpool.tile([P, t_main], F32, tag="nrecip")
    nc.vector.reciprocal(nrecip[:, :T], nssum[:, :T])

    # ---- w = relu(alpha - mdist/S) * vmb ----
    w = statpool.tile([P, t_main * K], F32, tag="w")
    for t in range(T):
        nc.scalar.activation(
            out=w[:, t * K : (t + 1) * K],
            in_=mdist[:, t * K : (t + 1) * K],
            func=ACTF.Relu,
            bias=alpha_t[:, :],
            scale=nrecip[:, t : t + 1],
        )
    nc.vector.tensor_mul(w[:, :KT], w[:, :KT], vmb[:, :KT])

    wsum = statpool.tile([P, t_main], F32, tag="wsum")
    nc.vector.tensor_reduce(
        out=wsum[:, :T],
        in_=w[:, :KT].rearrange("p (t k) -> p t k", k=K),
        axis=AX.X,
        op=ALU.add,
    )

    # ---- wdiff = diff * w (in place), reduce over k ----
    w_bc = w[:, :KT].rearrange("p (t k) -> p t k", k=K).to_broadcast((P, T, K, C))
    nc.vector.tensor_tensor(out=d4, in0=d4, in1=w_bc, op=ALU.mult)
    wdsum = statpool.tile([P, t_main * C], F32, tag="wdsum")
    g_kred = _ap(dbuf[:], [[K * C, T], [1, C], [C, K]])
    nc.vector.tensor_reduce(out=wdsum[:, : T * C], in_=g_kred, axis=AX.X, op=ALU.add)

    # ---- aflow = wdsum + wsum * lv  -> cat[:, t, 0:C] ----
    for t in range(T):
        nc.vector.scalar_tensor_tensor(
            out=cat[:, t, 0:C],
            in0=cat[:, t, C : 2 * C],
            scalar=wsum[:, t : t + 1],
            in1=wdsum[:, t * C : (t + 1) * C],
            op0=ALU.mult,
            op1=ALU.add,
        )

    # ---- linear layer + relu per sub-tile ----
    outsb = outpool.tile([P, t_main * C], F32, tag="outsb")
    for t in range(T):
        ctps = tps.tile([CAT, P], F32, tag="ctps")
        nc.tensor.transpose(out=ctps[:], in_=cat[:, t, 0:CAT], identity=ident[:])
        ctsb = ctpool.tile([CAT, P], F32, tag="ctsb")
        nc.scalar.copy(ctsb[:], ctps[:])
        ops = mps.tile([P, C], F32, tag="ops")
        nc.tensor.matmul(out=ops[:], lhsT=ctsb[:], rhs=wb_sb[:], start=True, stop=True)
        nc.scalar.activation(out=outsb[:, t * C : (t + 1) * C], in_=ops[:], func=ACTF.Relu)

    nc.scalar.dma_start(
        out=out_d[base : base + rows, :].rearrange("(p t) c -> p (t c)", t=T),
        in_=outsb[:, : T * C],
    )


_PROGRAM_CACHE = {}


def _get_program(per_core, table_rows, alpha, beta, t_main=T_MAIN):
    key = (per_core, table_rows, float(alpha), float(beta), t_main)
    if key not in _PROGRAM_CACHE:
        _PROGRAM_CACHE[key] = build_program(per_core, table_rows, alpha, beta, t_main)
    return _PROGRAM_CACHE[key]


def _shard_inputs(lv, hidden_state, W, b_lin, b_aflow, alpha, beta, neighbor_idx):
    """Pad + shard on host. Returns in_maps for the 8 cores."""
    lv = np.ascontiguousarray(np.asarray(lv, dtype=np.float32))
    hs = np.ascontiguousarray(np.asarray(hidden_state, dtype=np.float32))
    idx = np.ascontiguousarray(np.asarray(neighbor_idx, dtype=np.int32))
    W = np.asarray(W, dtype=np.float32)
    b_lin = np.asarray(b_lin, dtype=np.float32)
    b_aflow = np.asarray(b_aflow, dtype=np.float32)

    n = lv.shape[0]
    pad = PAD_N - n
    lv_p = np.concatenate([lv, np.zeros((pad, C), np.float32)], axis=0)
    idx_p = np.concatenate([idx, np.zeros((pad, K), np.int32)], axis=0)

    # fold b_aflow into the linear layer: aflow' = aflow_nobias, and
    # cat @ W + b_lin == [aflow', lv, 1] @ [[W],[b_lin + b_aflow @ W_a]]
    # where W_a is the first C rows of W (the aflow part).
    bias_row = b_lin + b_aflow @ W[:C, :]
    wb = np.concatenate([W, bias_row[None, :]], axis=0).astype(np.float32)

    in_maps = []
    for i in range(NCORES):
        s = i * PER_CORE
        e = s + PER_CORE
        in_maps.append(
            {
                "lv": lv_p[s:e],
                "hs": hs,
                "nidx": idx_p[s:e],
                "wb": wb,
            }
        )
    return in_maps


def kernel(lv, hidden_state, W, b_lin, b_aflow, alpha, beta, neighbor_idx):
    n = np.asarray(lv).shape[0]
    in_maps = _shard_inputs(lv, hidden_state, W, b_lin, b_aflow, alpha, beta, neighbor_idx)
    nc = _get_program(PER_CORE, np.asarray(hidden_state).shape[0], float(alpha), float(beta))
    res = run_bass_kernel_spmd(nc, in_maps, core_ids=list(range(NCORES)))
    out = np.concatenate([res.results[i]["out"] for i in range(NCORES)], axis=0)
    return out[:n]

